# revision 27
# baseline (speedup 1.0000x reference)
"""Trainium2 Bass kernel for a single transformer block (nn_Block_3212635537783).

Reference computation (B=4, T=2048, C=768, H=12, D=64):
    q/k/v per-head projections of x; scores[t,s] = k[t]@q[s]/sqrt(C) with
    causal mask (s <= t), softmax over s; a[t] = sum_s w[t,s] v[s];
    x = LN1(x + a); x = LN2(x + gelu(x@W1 + b1)@W2 + b2)

Sharding: 8 cores = 4 batches x 2 token-interleaved halves. Core (b, g)
owns rows {g, g+2, ...} of batch b. The stride-2 interleave keeps the
causal workload balanced AND the SPMD program identical across cores
(only input data differs; the +-1 row causal boundary lives in a tiny
per-core mask tile).

On-chip layout is fully "transposed": activations are [C, tokens]
(feature dim on partitions) so attention, layernorm and the MLP never
need an on-chip transpose. Matmul inputs are bf16 (fp32 accumulation).

Per-token scalars (softmax denominators, LN statistics) are handled
without any single-lane DVE reciprocal (measured 3.3us per [1,512] --
it was the dominant stall source): 1/d = exp(-ln d), with the negation
folded into the ones-matmul that broadcasts the per-token row across
partitions, and ln/exp sharing one ACT table set with attention's exp.
"""

import sys
import types

import numpy as np
import ml_dtypes

B, T, C, H, D = 4, 2048, 768, 12, 64
F = 4 * C            # 3072
P = 128              # partitions
OT = T // 2          # owned tokens per core (1024)
NB_C = C // P        # 6 c-chunks
NB_F = F // P        # 24 hidden chunks
NPAIR = H // 2       # 6 head-pair chunks
EPS = 1e-5
SCALE = float(1.0 / np.sqrt(np.float32(C)))
N_CORES = 8
HG = 4               # heads per attention group
N_HG = H // HG       # 3 groups

BF16 = ml_dtypes.bfloat16

_compiled = {}


# --------------------------------------------------------------------------
# environment patches (must live in kernel.py: the grader imports only this
# file). Idempotent.
# --------------------------------------------------------------------------

def _patch_tile_drain():
    """This walrus build rejects >1 sync-wait command on the final Tile
    drain CTRL instruction. Spread the drain's waits across chained
    sync-engine nops (same engine => program order preserved; the
    all-engine barrier after them still gates the semaphore clears)."""
    import concourse.tile as tile_mod
    import concourse.mybir as mybir

    if getattr(tile_mod.TileContext, "_drain_patched", False):
        return

    def patched(self, tick_clock, wait_clock):
        from concourse.vector_clock import ScopedClock

        drain_inst = self.nc.sync.drain()
        wait_clock.add_sem_waits(
            drain_inst.ins, ScopedClock({None: tick_clock.global_clock})
        )
        si = drain_inst.ins.sync_info
        waits = list(si.on_wait) if si else []
        MAXW = 1
        if len(waits) > MAXW:
            si.on_wait = waits[:MAXW]
            rest = waits[MAXW:]
            while rest:
                nop = self.nc.sync.nop(nofuse=True)
                chunk, rest = rest[:MAXW], rest[MAXW:]
                nsi = nop.ins.sync_info
                if nsi is None:
                    nop.ins.sync_info = mybir.SyncInfo(on_wait=chunk, on_update=[])
                else:
                    nsi.on_wait = list(nsi.on_wait) + chunk
        self.nc.all_engine_barrier()
        assert self.sems is not None
        popped = self.nc._tile_sem_poison_stack.pop()
        assert popped is self._sem_poison
        self.nc.clear_and_free_semaphores(list(self.sems.allocated().values()))
        self.nc.all_engine_barrier()

    tile_mod.TileContext._drain_and_barrier = patched
    tile_mod.TileContext._drain_patched = True


def _patch_profile_hook():
    """Optional: register the axon NTFF profiling hook so trace=True works
    (used for timing; harmless no-op if unavailable)."""
    if "antenv.axon_hooks" in sys.modules:
        return
    try:
        sys.path.insert(0, "/root/.axon_site")
        from trn_agent_boot.trn_boot import _ntff_profile_via_ctypes

        hook = _ntff_profile_via_ctypes("/opt/axon/libaxon_pjrt.so")
        mod = types.ModuleType("antenv.axon_hooks")
        mod.get_axon_ntff_profile_hook = lambda: hook
        mod.set_axon_ntff_profile_hook = lambda h: None
        sys.modules["antenv.axon_hooks"] = mod
        import concourse.bass_utils as bu

        bu.upload_artifacts = lambda tmpdir: "local://" + tmpdir
    except Exception:
        pass


# --------------------------------------------------------------------------
# program construction (shared by all 8 cores; SPMD over input data)
# --------------------------------------------------------------------------

def _build_nc():
    import contextlib

    import concourse.bass as bass
    import concourse.mybir as mybir
    from concourse.tile import TileContext

    f32 = mybir.dt.float32
    f32r = mybir.dt.float32r
    bf16 = mybir.dt.bfloat16
    fp8 = mybir.dt.float8e4
    DR = mybir.MatmulPerfMode.DoubleRow
    ALU = mybir.AluOpType
    AF = mybir.ActivationFunctionType

    nc = bass.Bass()

    # ---- DRAM I/O ----
    xP8 = nc.declare_dram_parameter("xP8", [N_HG, P, 2 * T], fp8, isOutput=False)
    xoP8 = nc.declare_dram_parameter("xoP8", [N_HG, P, 2 * OT], fp8, isOutput=False)
    xTo16 = nc.declare_dram_parameter("xTo16", [C, OT], bf16, isOutput=False)
    wqL = nc.declare_dram_parameter("wqL", [NPAIR, P, NB_C * P], fp8, isOutput=False)
    wkL = nc.declare_dram_parameter("wkL", [NPAIR, P, NB_C * P], fp8, isOutput=False)
    wvL = nc.declare_dram_parameter("wvL", [N_HG, P, NB_C * HG * D], fp8, isOutput=False)
    w1 = nc.declare_dram_parameter("w1", [C, F], bf16, isOutput=False)
    w2 = nc.declare_dram_parameter("w2", [F, C], bf16, isOutput=False)
    b1r = nc.declare_dram_parameter("b1r", [P, NB_F], f32, isOutput=False)
    b2r = nc.declare_dram_parameter("b2r", [P, NB_C], f32, isOutput=False)
    g1r = nc.declare_dram_parameter("g1r", [P, NB_C], f32, isOutput=False)
    be1r = nc.declare_dram_parameter("be1r", [P, NB_C], f32, isOutput=False)
    g2r = nc.declare_dram_parameter("g2r", [P, NB_C], f32, isOutput=False)
    be2r = nc.declare_dram_parameter("be2r", [P, NB_C], f32, isOutput=False)
    cmask = nc.declare_dram_parameter("cmask", [P, 64], fp8, isOutput=False)
    outT = nc.declare_dram_parameter("outT", [C, OT], f32, isOutput=True)

    xP8_t = xP8[:].rearrange("j p (i t) -> j p i t", i=2)
    xoP8_t = xoP8[:].rearrange("j p (i t) -> j p i t", i=2)
    xTo16_t = xTo16[:].rearrange("(n p) t -> n p t", p=P)
    w1_t = w1[:].rearrange("(n p) f -> n p f", p=P)
    w2_t = w2[:].rearrange("(n p) c -> n p c", p=P)
    outT_t = outT[:].rearrange("(n p) t -> n p t", p=P)

    def r(ap):
        """bitcast fp32 matmul operands to fp32r (full-rate at N>=256)"""
        return ap.bitcast(f32r)

    with TileContext(nc) as tc, contextlib.ExitStack() as ctx:
        const = ctx.enter_context(tc.tile_pool(name="const", bufs=1))
        p_a = ctx.enter_context(tc.tile_pool(name="attn_a", bufs=1))
        p_mlpw = ctx.enter_context(tc.tile_pool(name="mlpw", bufs=1))
        import contextlib as _ctl
        att_stack = _ctl.ExitStack()
        p_xt = att_stack.enter_context(tc.tile_pool(name="xt", bufs=1))
        p_w = att_stack.enter_context(tc.tile_pool(name="wqkv", bufs=1))
        p_qk = att_stack.enter_context(tc.tile_pool(name="qk", bufs=1))
        p_v = att_stack.enter_context(tc.tile_pool(name="vv", bufs=1))
        p_ps = att_stack.enter_context(
            tc.tile_pool(name="ps", bufs=1, space="PSUM")
        )
        p_e = att_stack.enter_context(tc.tile_pool(name="et", bufs=1))
        p_dn = att_stack.enter_context(tc.tile_pool(name="dn", bufs=1))

        # ---- constants ----
        ones_k = const.tile([P, 1], bf16, tag="ones_k", name="ones_k")
        nc.vector.memset(ones_k, 1.0)
        stage_k = const.tile([P, 1], f32, tag="stage_k", name="stage_k")
        nc.vector.memset(stage_k, 1.0)
        stage_r = const.tile([1, P], f32, tag="stage_r", name="stage_r")
        nc.vector.memset(stage_r, 1.0)
        with nc.allow_low_precision(reason="f32r constants for bcast matmuls"):
            ones_k32 = const.tile([P, 1], f32r, tag="ones_k32", name="ones_k32")
            nc.vector.tensor_copy(ones_k32, stage_k)
            ones_row = const.tile([1, P], f32r, tag="ones_row", name="ones_row")
            nc.vector.tensor_copy(ones_row, stage_r)
            neg_row = const.tile([1, P], f32r, tag="neg_row", name="neg_row")
            nc.vector.tensor_scalar_mul(neg_row, stage_r, -1.0)
            negh_row = const.tile([1, P], f32r, tag="negh_row", name="negh_row")
            nc.vector.tensor_scalar_mul(negh_row, stage_r, -0.5)
        eps_t = const.tile([1, 1], f32, tag="eps", name="eps_t")
        nc.vector.memset(eps_t, EPS)
        msk = const.tile([P, 64], fp8, tag="msk", name="msk")
        nc.sync.dma_start(out=msk, in_=cmask[:])
        msk2 = bass.AP(
            tensor=msk.tensor, offset=msk.offset,
            ap=[list(msk.ap[0]), [0, 2], list(msk.ap[1])],
        )
        sb_b1 = const.tile([P, NB_F], f32, tag="b1", name="sb_b1")
        nc.sync.dma_start(out=sb_b1, in_=b1r[:])
        sb_b2 = const.tile([P, NB_C], f32, tag="b2", name="sb_b2")
        nc.sync.dma_start(out=sb_b2, in_=b2r[:])
        sb_g1 = const.tile([P, NB_C], f32, tag="g1", name="sb_g1")
        nc.sync.dma_start(out=sb_g1, in_=g1r[:])
        sb_be1 = const.tile([P, NB_C], f32, tag="be1", name="sb_be1")
        nc.sync.dma_start(out=sb_be1, in_=be1r[:])
        sb_g2 = const.tile([P, NB_C], f32, tag="g2", name="sb_g2")
        nc.sync.dma_start(out=sb_g2, in_=g2r[:])
        sb_be2 = const.tile([P, NB_C], f32, tag="be2", name="sb_be2")
        nc.sync.dma_start(out=sb_be2, in_=be2r[:])

        # ---- persistent activations (fp8 pair layout for DoubleRow) ----
        xp8 = [
            p_xt.tile([P, 2, T], fp8, tag=f"xp8_{j}", name=f"xp8_{j}")
            for j in range(N_HG)
        ]
        xo8 = [
            p_xt.tile([P, 2, OT], fp8, tag=f"xo8_{j}", name=f"xo8_{j}")
            for j in range(N_HG)
        ]
        sb_xto16 = []
        for k in range(NB_C):
            # xto16 lives in the persistent pool: the MLP-phase residual
            # adds still read it after the attention pools close.
            t3 = p_a.tile([P, OT], bf16, tag=f"xto16_{k}", name=f"xto16_{k}")
            nc.sync.dma_start(out=t3, in_=xTo16_t[k])
            sb_xto16.append(t3)
        # fine-grained loads: the first projection matmul only needs the
        # first 512-column slice of each pair tile.
        for t4 in range(T // 512):
            for j in range(N_HG):
                nc.sync.dma_start(
                    out=xp8[j][:, :, t4 * 512 : (t4 + 1) * 512],
                    in_=xP8_t[j][:, :, t4 * 512 : (t4 + 1) * 512],
                )
        for t2 in range(OT // 512):
            for j in range(N_HG):
                nc.sync.dma_start(
                    out=xo8[j][:, :, t2 * 512 : (t2 + 1) * 512],
                    in_=xoP8_t[j][:, :, t2 * 512 : (t2 + 1) * 512],
                )

        def xt_own(k, tb):
            """own-token columns of chunk k, token block tb."""
            return sb_xto16[k][:, tb * 512 : (tb + 1) * 512]

        # attention output a^T, bf16 [128, OT] per pair-chunk
        sb_a = [
            p_a.tile([P, OT], bf16, tag=f"a{pc}", name=f"a{pc}")
            for pc in range(NPAIR)
        ]

        # MLP weight tiles: allocate now (address space), DMA after the
        # first projection wave's weights are queued.
        sb_w1 = [
            p_mlpw.tile([P, F], bf16, tag=f"w1_{k}", name=f"w1_{k}")
            for k in range(NB_C)
        ]
        sb_w2 = [
            p_mlpw.tile([P, C], bf16, tag=f"w2_{m}", name=f"w2_{m}")
            for m in range(NB_F)
        ]

        # ============================================================
        # Phase A: attention, in head groups of HG. Pools are global so
        # group hg+1's projections overlap group hg's attention tail.
        # ============================================================
        q_t, k_t = {}, {}
        for hg in range(N_HG):
            pcs = [hg * (HG // 2) + i for i in range(HG // 2)]
            heads = [2 * pc + j for pc in pcs for j in range(2)]

            _sc_p = nc.enter_named_scope(f"proj{hg}", False)
            # ---- projections for this group ----
            for pc in pcs:
                q_t[pc] = p_qk.tile(
                    [P, T], bf16, tag=f"q{pc % 2}", bufs=2, name=f"q{pc}"
                )
                k_t[pc] = p_qk.tile(
                    [P, OT], bf16, tag=f"k{pc % 2}", bufs=2, name=f"k{pc}"
                )
                wqt = p_w.tile([P, NB_C * P], fp8, tag="wq", bufs=2, name="wqt")
                nc.sync.dma_start(out=wqt, in_=wqL[pc])
                for t4 in range(T // 512):
                    ps = p_ps.tile([P, 2, 512], f32, tag="ps", bufs=2, name="ps_prj")
                    for j in range(N_HG):
                        nc.tensor.matmul(
                            ps[:, 0, :],
                            wqt[:, 2 * j * P : 2 * (j + 1) * P].rearrange(
                                "p (i c) -> p i c", i=2
                            ),
                            xp8[j][:, :, t4 * 512 : (t4 + 1) * 512],
                            start=(j == 0),
                            stop=(j == N_HG - 1),
                            perf_mode=DR,
                        )
                    nc.vector.tensor_copy(
                        q_t[pc][:, t4 * 512 : (t4 + 1) * 512], ps[:, 0, :]
                    )
                wkt = p_w.tile([P, NB_C * P], fp8, tag="wk", bufs=2, name="wkt")
                nc.sync.dma_start(out=wkt, in_=wkL[pc])
                for t2 in range(OT // 512):
                    ps = p_ps.tile([P, 2, 512], f32, tag="ps", bufs=2, name="ps_prk")
                    for j in range(N_HG):
                        nc.tensor.matmul(
                            ps[:, 0, :],
                            wkt[:, 2 * j * P : 2 * (j + 1) * P].rearrange(
                                "p (i c) -> p i c", i=2
                            ),
                            xo8[j][:, :, t2 * 512 : (t2 + 1) * 512],
                            start=(j == 0),
                            stop=(j == N_HG - 1),
                            perf_mode=DR,
                        )
                    nc.vector.tensor_copy(
                        k_t[pc][:, t2 * 512 : (t2 + 1) * 512], ps[:, 0, :]
                    )

            # v projection: full T, DoubleRow, into fp8 [128, 2, HG, 65]
            # s-chunk-pair tiles (the AV DoubleRow stationary layout).
            wvt = p_w.tile(
                [P, NB_C * HG * D], fp8, tag="wv", bufs=2, name="wvt"
            )
            nc.sync.dma_start(out=wvt, in_=wvL[hg])
            v8 = []
            for jp in range(T // P // 2):
                # head slot padded 65 -> 68 so the DoubleRow pair stride
                # (HG*68 = 272 B) meets the 16 B LDWEIGHTS alignment rule.
                vt = p_v.tile(
                    [P, 2, HG, 68], fp8, tag=f"v8_{jp}", bufs=2, name=f"v8_{jp}"
                )
                nc.vector.memset(vt[:, :, :, 64:65], 1.0)
                v8.append(vt)
            for sc in range(T // P):
                ps = p_ps.tile([P, 2, 512], f32, tag="ps", bufs=2, name="ps_v")
                for j in range(N_HG):
                    nc.tensor.matmul(
                        ps[:, 0, 0 : HG * D],
                        xp8[j][:, :, sc * P : (sc + 1) * P],
                        wvt[:, 2 * j * HG * D : 2 * (j + 1) * HG * D].rearrange(
                            "p (i c) -> p i c", i=2
                        ),
                        start=(j == 0),
                        stop=(j == N_HG - 1),
                        perf_mode=DR,
                    )
                nc.scalar.activation(
                    out=v8[sc // 2][:, sc % 2, :, 0:64],
                    in_=ps[:, 0, 0 : HG * D].rearrange("p (h d) -> p h d", h=HG),
                    func=AF.Identity,
                    scale=0.125,
                )

            nc.leave_named_scope(f"proj{hg}", _sc_p[0], False)
            if hg == 1:
                for k in range(NB_C):
                    nc.sync.dma_start(out=sb_w1[k], in_=w1_t[k])
                for m in range(NB_F):
                    nc.sync.dma_start(out=sb_w2[m], in_=w2_t[m])
            _sc_a = nc.enter_named_scope(f"attn{hg}", False)
            # ---- attention ----
            for tb in range(2):
                nsc = 8 * tb + 8          # s-chunks for this own-block
                av = {}
                for h in heads:
                    av[h] = p_ps.tile(
                        [P, 512], f32, tag=f"av{h % HG}", bufs=1, name=f"av{h}"
                    )
                npp = nsc // 2
                for jp in range(npp):
                    sc0, sc1 = 2 * jp, 2 * jp + 1
                    c0e = max(0, 64 * sc0 - 512 * tb)
                    c0o = max(0, 64 * sc1 - 512 * tb)
                    for pc in pcs:
                        et8 = p_e.tile(
                            [P, 2, 2, 512], fp8, tag="exp", bufs=4, name="et8"
                        )
                        for i, (sc, c0) in enumerate(
                            ((sc0, c0e), (sc1, c0o))
                        ):
                            ps = p_ps.tile(
                                [P, 2, 512], f32, tag="ps", bufs=2, name="ps_sc"
                            )
                            for par in range(2):
                                nc.tensor.matmul(
                                    ps[:, par, c0:512],
                                    q_t[pc][par * 64 : par * 64 + 64,
                                            sc * P : (sc + 1) * P],
                                    k_t[pc][par * 64 : par * 64 + 64,
                                            tb * 512 + c0 : (tb + 1) * 512],
                                    start=True,
                                    stop=True,
                                )
                            nc.scalar.activation(
                                out=et8[:, :, i, c0:512],
                                in_=ps[:, :, c0:512],
                                func=AF.Exp,
                                scale=SCALE / 64.0,
                            )
                        if sc0 >= 8 * tb:   # causal boundary stripes
                            nc.vector.tensor_tensor(
                                et8[:, :, 0, c0e : c0e + 64],
                                et8[:, :, 0, c0e : c0e + 64],
                                msk2[:, :, 0:64],
                                ALU.mult,
                            )
                            nc.vector.memset(
                                et8[:, :, 1, c0e : c0e + 64], 0.0
                            )
                            nc.vector.tensor_tensor(
                                et8[:, :, 1, c0o : c0o + 64],
                                et8[:, :, 1, c0o : c0o + 64],
                                msk2[:, :, 0:64],
                                ALU.mult,
                            )
                        for par in range(2):
                            h = 2 * pc + par
                            jj = heads.index(h)
                            nc.tensor.matmul(
                                av[h][0:65, c0e:512],
                                v8[jp][:, :, jj, 0:65],
                                et8[:, par, :, c0e:512],
                                start=(jp == 0),
                                stop=(jp == npp - 1),
                                perf_mode=DR,
                            )
                # normalize: 1/den = exp(-ln den), negation folded into
                # the broadcast matmul; single multiply reads AV psum.
                for h in heads:
                    pc, par = h // 2, h % 2
                    nl = p_dn.tile([1, 512], f32r, tag="nl", bufs=4, name="nl")
                    nc.scalar.activation(
                        out=nl, in_=av[h][64:65, 0:512], func=AF.Ln
                    )
                    rb = p_ps.tile(
                        [P, 2, 512], f32, tag="ps", bufs=2, name="rb"
                    )
                    nc.tensor.matmul(
                        rb[0:64, 0, :], neg_row[:, 0:64], nl,
                        start=True, stop=True,
                    )
                    rec = p_dn.tile(
                        [64, 512], f32, tag="rec", bufs=4, name="rec"
                    )
                    nc.scalar.activation(
                        out=rec, in_=rb[0:64, 0, :], func=AF.Exp
                    )
                    nc.vector.tensor_tensor(
                        sb_a[pc][par * 64 : par * 64 + 64,
                                 tb * 512 : (tb + 1) * 512],
                        av[h][0:64, 0:512],
                        rec,
                        ALU.mult,
                    )

            nc.leave_named_scope(f"attn{hg}", _sc_a[0], False)

        att_stack.close()   # free attention pools before the MLP pools open

        # ============================================================
        # Phase B: residual + LN1 + MLP + residual + LN2
        # ============================================================
        with contextlib.ExitStack() as mctx:
            mctx.enter_context(nc.named_scope("mlp"))
            p_r1 = mctx.enter_context(tc.tile_pool(name="r1", bufs=1))
            p_ln = mctx.enter_context(tc.tile_pool(name="ln", bufs=1))
            p_tmp = mctx.enter_context(tc.tile_pool(name="tmp", bufs=1))
            p_st = mctx.enter_context(tc.tile_pool(name="st", bufs=1))
            p_psm = mctx.enter_context(
                tc.tile_pool(name="psm", bufs=1, space="PSUM")
            )
            p_h = mctx.enter_context(tc.tile_pool(name="hsb", bufs=1))
            p_out = mctx.enter_context(tc.tile_pool(name="outp", bufs=1))

            r1 = [
                p_r1.tile([P, OT], f32r, tag=f"r1_{c}", name=f"r1_{c}")
                for c in range(NB_C)
            ]
            ln1 = [
                p_ln.tile([P, OT], bf16, tag=f"ln1_{c}", name=f"ln1_{c}")
                for c in range(NB_C)
            ]

            def layer_norm_T(src_tiles, out_cb):
                """transposed LN over the partition (c) dim. Stats via
                f32r/bf16 ones-matmuls; rsqrt as exp(-0.5 ln(var+eps));
                broadcasts via rank-1 matmuls. src_tiles: 6 x [128,512]
                f32 SBUF views."""
                mu_ps = p_psm.tile([1, 512], f32, tag="lnst", bufs=2, name="mu_ps")
                sq_ps = p_psm.tile([1, 512], f32, tag="lnst", bufs=2, name="sq_ps")
                for c in range(NB_C):
                    s = p_tmp.tile([P, 512], bf16, tag="sqt", bufs=1, name="sqt")
                    nc.vector.tensor_tensor(s, src_tiles[c], src_tiles[c], ALU.mult)
                    nc.tensor.matmul(
                        mu_ps, ones_k32, src_tiles[c],
                        start=(c == 0), stop=(c == NB_C - 1),
                    )
                    nc.tensor.matmul(
                        sq_ps, ones_k, s,
                        start=(c == 0), stop=(c == NB_C - 1),
                    )
                mun = p_st.tile([1, 512], f32r, tag="mun", bufs=1, name="mun")
                nc.scalar.activation(
                    out=mun, in_=mu_ps, func=AF.Identity, scale=1.0 / C
                )
                m2 = p_st.tile([1, 512], f32, tag="m2", bufs=1, name="m2")
                nc.scalar.activation(
                    out=m2, in_=mu_ps, func=AF.Square, scale=1.0 / C
                )
                var = p_st.tile([1, 512], f32, tag="var", bufs=1, name="var")
                nc.vector.tensor_scalar_mul(var, sq_ps, 1.0 / C)
                nc.vector.tensor_tensor(var, var, m2, ALU.subtract)
                lv = p_st.tile([1, 512], f32r, tag="lv", bufs=1, name="lv")
                nc.scalar.activation(
                    out=lv, in_=var, func=AF.Ln, bias=eps_t, scale=1.0
                )
                mu_b = p_psm.tile([P, 512], f32, tag="lnbc", bufs=2, name="mu_b")
                nc.tensor.matmul(mu_b, ones_row, mun, start=True, stop=True)
                rs_ps = p_psm.tile([P, 512], f32, tag="lnbc", bufs=2, name="rs_ps")
                nc.tensor.matmul(rs_ps, negh_row, lv, start=True, stop=True)
                rs_b = p_tmp.tile([P, 512], f32, tag="rs_b", bufs=2, name="rs_b")
                nc.scalar.activation(out=rs_b, in_=rs_ps, func=AF.Exp)
                for c in range(NB_C):
                    d1 = p_tmp.tile([P, 512], f32, tag="d1", bufs=2, name="d1")
                    nc.vector.tensor_tensor(d1, src_tiles[c], mu_b, ALU.subtract)
                    nc.vector.tensor_tensor(d1, d1, rs_b, ALU.mult)
                    out_cb(c, d1)

            # residual + LN1 for BOTH halves first: LN1(tb=1)'s DVE work
            # then overlaps MLP(tb=0)'s matmuls.
            for tb in range(2):
                sl = slice(tb * 512, (tb + 1) * 512)
                r1v = []
                for c in range(NB_C):
                    with nc.allow_low_precision(reason="f32r residual store"):
                        nc.vector.tensor_tensor(
                            r1[c][:, sl], xt_own(c, tb), sb_a[c][:, sl], ALU.add
                        )
                    r1v.append(r1[c][:, sl])

                def ln1_out(c, d2, _sl=sl):
                    nc.vector.tensor_scalar(
                        out=ln1[c][:, _sl], in0=d2,
                        scalar1=sb_g1[:, c : c + 1], scalar2=sb_be1[:, c : c + 1],
                        op0=ALU.mult, op1=ALU.add,
                    )
                    # f32 copy for the post-LN residual (r1 is dead: reuse)
                    with nc.allow_low_precision(reason="f32r residual store"):
                        nc.vector.tensor_scalar(
                            out=r1[c][:, _sl], in0=d2,
                            scalar1=sb_g1[:, c : c + 1], scalar2=sb_be1[:, c : c + 1],
                            op0=ALU.mult, op1=ALU.add,
                        )

                layer_norm_T(r1v, ln1_out)

            # MLP pass 1, both halves per m-tile: the stationary W1 slice
            # is shared by consecutive matmuls (halves the LDWEIGHTS
            # serialization) and gelu covers [P, OT] per tile.
            h_sb = []
            for m in range(NB_F):
                hp = {}
                for tb in range(2):
                    hp[tb] = p_psm.tile(
                        [P, 512], f32, tag="h_ps", bufs=2, name="h_ps"
                    )
                for k in range(NB_C):
                    for tb in range(2):
                        nc.tensor.matmul(
                            hp[tb],
                            sb_w1[k][:, m * P : (m + 1) * P],
                            ln1[k][:, tb * 512 : (tb + 1) * 512],
                            start=(k == 0),
                            stop=(k == NB_C - 1),
                        )
                hs = p_h.tile([P, OT], bf16, tag=f"h{m}", name=f"h{m}")
                for tb in range(2):
                    nc.scalar.activation(
                        out=hs[:, tb * 512 : (tb + 1) * 512], in_=hp[tb],
                        func=AF.Gelu, bias=sb_b1[:, m : m + 1], scale=1.0,
                    )
                h_sb.append(hs)

            for tb in range(2):
                sl = slice(tb * 512, (tb + 1) * 512)

                # MLP pass 2: y[c] = sum_m W2[m,c].T h[m];  r2 = y + b2 + r1
                r2v = []
                for c in range(NB_C):
                    y_ps = p_psm.tile([P, 512], f32, tag="y_ps", bufs=2, name="y_ps")
                    for m in range(NB_F):
                        nc.tensor.matmul(
                            y_ps,
                            sb_w2[m][:, c * P : (c + 1) * P],
                            h_sb[m][:, sl],
                            start=(m == 0),
                            stop=(m == NB_F - 1),
                        )
                    y_sb = p_tmp.tile([P, 512], f32, tag="y_sb", bufs=2, name="y_sb")
                    nc.vector.tensor_scalar(
                        out=y_sb, in0=y_ps,
                        scalar1=sb_b2[:, c : c + 1], scalar2=None,
                        op0=ALU.add,
                    )
                    with nc.allow_low_precision(reason="f32r residual store"):
                        nc.vector.tensor_tensor(
                            r1[c][:, sl], y_sb, r1[c][:, sl], ALU.add
                        )
                    r2v.append(r1[c][:, sl])

                def ln2_out(c, d2, _sl=sl):
                    o = p_out.tile([P, 512], f32, tag="o", bufs=2, name="o")
                    nc.vector.tensor_scalar(
                        out=o, in0=d2,
                        scalar1=sb_g2[:, c : c + 1], scalar2=sb_be2[:, c : c + 1],
                        op0=ALU.mult, op1=ALU.add,
                    )
                    nc.sync.dma_start(out=outT_t[c][:, _sl], in_=o)

                layer_norm_T(r2v, ln2_out)

    return nc


def _spill_excess_waits(nc, maxw=2):
    """walrus (this build) caps sync-wait commands per instruction. Move
    excess waits onto freshly inserted same-engine nops placed immediately
    before the over-limit instruction (same engine stream => the waits
    still complete before it executes)."""
    import copy

    import concourse.bass as bass
    import concourse.mybir as mybir

    scratch = bass.Bass()
    tpl = scratch.sync.nop(nofuse=True).ins
    ctr = [0]

    def mknop(engine, waits):
        n = copy.deepcopy(tpl)
        ctr[0] += 1
        n.name = f"I-spill{ctr[0]}"
        n.engine = engine
        n.sync_info = mybir.SyncInfo(on_wait=list(waits), on_update=[])
        return n

    fn = nc.m.functions[0]
    for bb in fn.blocks:
        changed = False
        out = []
        for inst in bb.instructions:
            si = inst.sync_info
            waits = list(si.on_wait) if si and si.on_wait else []
            nupd = len(si.on_update) if si and si.on_update else 0
            lim = max(0, maxw - nupd)   # waits + updates <= maxw total
            if len(waits) > lim:
                keep = waits[-lim:] if lim else []
                rest = waits[: len(waits) - lim]
                while rest:
                    chunk, rest = rest[:1], rest[1:]
                    out.append(mknop(inst.engine, chunk))
                si.on_wait = keep
                changed = True
            out.append(inst)
        if changed:
            bb.instructions = out


def _get_nc():
    if "nc" not in _compiled:
        _patch_tile_drain()
        _patch_profile_hook()
        nc = _build_nc()
        _spill_excess_waits(nc, maxw=2)
        _compiled["nc"] = nc
    return _compiled["nc"]


# --------------------------------------------------------------------------
# host-side sharding
# --------------------------------------------------------------------------

E4 = ml_dtypes.float8_e4m3


def _q8(a):
    return np.clip(a, -240.0, 240.0).astype(E4)


def _make_in_maps(x, Wq, Wk, Wv, ln1_g, ln1_b, W1, b1, W2, b2, ln2_g, ln2_b):
    x = np.asarray(x, np.float32)
    wq_s = np.ascontiguousarray(
        np.asarray(Wq, np.float32).transpose(1, 0, 2).reshape(C, C)
    )
    wk_s = np.ascontiguousarray(
        np.asarray(Wk, np.float32).transpose(1, 0, 2).reshape(C, C)
    )
    wv_s = np.ascontiguousarray(
        np.asarray(Wv, np.float32).transpose(1, 0, 2).reshape(C, C)
    )
    # fp8 DoubleRow pair layouts (weights pre-scaled by 16, x by 1/2;
    # the 64x score factor is folded into the exp scale, the 8x v factor
    # into the v-copy scale).
    # wqL[pc, p, j*256 + i*128 + c2] = 16*wq[(2j+i)*128+p, pc*128+c2]
    wqL = np.ascontiguousarray(
        _q8(wq_s.reshape(N_HG, 2, P, NPAIR, P).transpose(3, 2, 0, 1, 4) * 16.0)
        .reshape(NPAIR, P, C)
    )
    wkL = np.ascontiguousarray(
        _q8(wk_s.reshape(N_HG, 2, P, NPAIR, P).transpose(3, 2, 0, 1, 4) * 16.0)
        .reshape(NPAIR, P, C)
    )
    # wvL[hg, p, j*512 + i*256 + c2] = 16*wv[(2j+i)*128+p, hg*256+c2]
    wvL = np.ascontiguousarray(
        _q8(wv_s.reshape(N_HG, 2, P, N_HG, HG * D).transpose(3, 2, 0, 1, 4) * 16.0)
        .reshape(N_HG, P, NB_C * HG * D)
    )
    w1b = np.asarray(W1, np.float32).astype(BF16)
    w2b = np.asarray(W2, np.float32).astype(BF16)
    b1r = np.ascontiguousarray(np.asarray(b1, np.float32).reshape(NB_F, P).T)
    b2r = np.ascontiguousarray(np.asarray(b2, np.float32).reshape(NB_C, P).T)
    g1r = np.ascontiguousarray(np.asarray(ln1_g, np.float32).reshape(NB_C, P).T)
    be1r = np.ascontiguousarray(np.asarray(ln1_b, np.float32).reshape(NB_C, P).T)
    g2r = np.ascontiguousarray(np.asarray(ln2_g, np.float32).reshape(NB_C, P).T)
    be2r = np.ascontiguousarray(np.asarray(ln2_b, np.float32).reshape(NB_C, P).T)

    in_maps = []
    for core in range(N_CORES):
        b, g = core // 2, core % 2
        xb = x[b]                                # [T, C]
        xTa = np.ascontiguousarray(xb.T)         # [C, T]
        own = np.arange(g, T, 2)
        xo = np.ascontiguousarray(xb[own].T)     # [C, OT]
        # fp8 pair layouts: xP8[j, p, i*T + t] = fp8(0.5*x[t, (2j+i)*128+p])
        xP8 = np.ascontiguousarray(
            _q8(xTa.reshape(N_HG, 2, P, T).transpose(0, 2, 1, 3) * 0.5)
            .reshape(N_HG, P, 2 * T)
        )
        xoP8 = np.ascontiguousarray(
            _q8(xo.reshape(N_HG, 2, P, OT).transpose(0, 2, 1, 3) * 0.5)
            .reshape(N_HG, P, 2 * OT)
        )
        ii = np.arange(P)[:, None]
        mm = np.arange(64)[None, :]
        cm = np.where(ii <= 2 * mm + g, 1.0, 0.0).astype(E4)
        in_maps.append(
            {
                "xP8": xP8,
                "xoP8": xoP8,
                "xTo16": xo.astype(BF16),
                "wqL": wqL,
                "wkL": wkL,
                "wvL": wvL,
                "w1": w1b,
                "w2": w2b,
                "b1r": b1r,
                "b2r": b2r,
                "g1r": g1r,
                "be1r": be1r,
                "g2r": g2r,
                "be2r": be2r,
                "cmask": cm,
            }
        )
    return in_maps


def _assemble(results):
    out = np.empty((B, T, C), np.float32)
    for core in range(N_CORES):
        b, g = core // 2, core % 2
        own = np.arange(g, T, 2)
        out[b, own, :] = results[core]["outT"].T
    return out


def kernel(_trace=False, **inputs):
    from concourse.bass_utils import run_bass_kernel_spmd

    nc = _get_nc()
    in_maps = _make_in_maps(**inputs)
    res = run_bass_kernel_spmd(nc, in_maps, list(range(N_CORES)), trace=_trace)
    out = _assemble(res.results)
    if _trace:
        return out, res
    return out


# revision 28
# speedup vs baseline: 1.2092x; 1.2092x over previous
"""Trainium2 Bass kernel for a single transformer block (nn_Block_3212635537783).

Reference computation (B=4, T=2048, C=768, H=12, D=64):
    q/k/v per-head projections of x; scores[t,s] = k[t]@q[s]/sqrt(C) with
    causal mask (s <= t), softmax over s; a[t] = sum_s w[t,s] v[s];
    x = LN1(x + a); x = LN2(x + gelu(x@W1 + b1)@W2 + b2)

Sharding: 8 cores = 4 batches x 2 token-interleaved halves. Core (b, g)
owns rows {g, g+2, ...} of batch b. The stride-2 interleave keeps the
causal workload balanced AND the SPMD program identical across cores
(only input data differs; the +-1 row causal boundary lives in a tiny
per-core mask tile).

On-chip layout is fully "transposed": activations are [C, tokens]
(feature dim on partitions) so attention, layernorm and the MLP never
need an on-chip transpose. Matmul inputs are bf16 (fp32 accumulation).

Per-token scalars (softmax denominators, LN statistics) are handled
without any single-lane DVE reciprocal (measured 3.3us per [1,512] --
it was the dominant stall source): 1/d = exp(-ln d), with the negation
folded into the ones-matmul that broadcasts the per-token row across
partitions, and ln/exp sharing one ACT table set with attention's exp.
"""

import sys
import types

import numpy as np
import ml_dtypes

B, T, C, H, D = 4, 2048, 768, 12, 64
F = 4 * C            # 3072
P = 128              # partitions
OT = T // 2          # owned tokens per core (1024)
NB_C = C // P        # 6 c-chunks
NB_F = F // P        # 24 hidden chunks
NPAIR = H // 2       # 6 head-pair chunks
EPS = 1e-5
SCALE = float(1.0 / np.sqrt(np.float32(C)))
N_CORES = 8
HG = 4               # heads per attention group
N_HG = H // HG       # 3 groups

BF16 = ml_dtypes.bfloat16

_compiled = {}


# --------------------------------------------------------------------------
# environment patches (must live in kernel.py: the grader imports only this
# file). Idempotent.
# --------------------------------------------------------------------------

def _patch_tile_drain():
    """This walrus build rejects >1 sync-wait command on the final Tile
    drain CTRL instruction. Spread the drain's waits across chained
    sync-engine nops (same engine => program order preserved; the
    all-engine barrier after them still gates the semaphore clears)."""
    import concourse.tile as tile_mod
    import concourse.mybir as mybir

    if getattr(tile_mod.TileContext, "_drain_patched", False):
        return

    def patched(self, tick_clock, wait_clock):
        from concourse.vector_clock import ScopedClock

        drain_inst = self.nc.sync.drain()
        wait_clock.add_sem_waits(
            drain_inst.ins, ScopedClock({None: tick_clock.global_clock})
        )
        si = drain_inst.ins.sync_info
        waits = list(si.on_wait) if si else []
        MAXW = 1
        if len(waits) > MAXW:
            si.on_wait = waits[:MAXW]
            rest = waits[MAXW:]
            while rest:
                nop = self.nc.sync.nop(nofuse=True)
                chunk, rest = rest[:MAXW], rest[MAXW:]
                nsi = nop.ins.sync_info
                if nsi is None:
                    nop.ins.sync_info = mybir.SyncInfo(on_wait=chunk, on_update=[])
                else:
                    nsi.on_wait = list(nsi.on_wait) + chunk
        self.nc.all_engine_barrier()
        assert self.sems is not None
        popped = self.nc._tile_sem_poison_stack.pop()
        assert popped is self._sem_poison
        self.nc.clear_and_free_semaphores(list(self.sems.allocated().values()))
        self.nc.all_engine_barrier()

    tile_mod.TileContext._drain_and_barrier = patched
    tile_mod.TileContext._drain_patched = True


def _patch_profile_hook():
    """Optional: register the axon NTFF profiling hook so trace=True works
    (used for timing; harmless no-op if unavailable)."""
    if "antenv.axon_hooks" in sys.modules:
        return
    try:
        sys.path.insert(0, "/root/.axon_site")
        from trn_agent_boot.trn_boot import _ntff_profile_via_ctypes

        hook = _ntff_profile_via_ctypes("/opt/axon/libaxon_pjrt.so")
        mod = types.ModuleType("antenv.axon_hooks")
        mod.get_axon_ntff_profile_hook = lambda: hook
        mod.set_axon_ntff_profile_hook = lambda h: None
        sys.modules["antenv.axon_hooks"] = mod
        import concourse.bass_utils as bu

        bu.upload_artifacts = lambda tmpdir: "local://" + tmpdir
    except Exception:
        pass


# --------------------------------------------------------------------------
# program construction (shared by all 8 cores; SPMD over input data)
# --------------------------------------------------------------------------

def _build_nc():
    import contextlib

    import concourse.bass as bass
    import concourse.mybir as mybir
    from concourse.tile import TileContext

    f32 = mybir.dt.float32
    f32r = mybir.dt.float32r
    bf16 = mybir.dt.bfloat16
    fp8 = mybir.dt.float8e4
    DR = mybir.MatmulPerfMode.DoubleRow
    ALU = mybir.AluOpType
    AF = mybir.ActivationFunctionType

    nc = bass.Bass()

    # ---- DRAM I/O ----
    xP8 = nc.declare_dram_parameter("xP8", [N_HG, P, 2 * T], fp8, isOutput=False)
    xoP8 = nc.declare_dram_parameter("xoP8", [N_HG, P, 2 * OT], fp8, isOutput=False)
    xTo16 = nc.declare_dram_parameter("xTo16", [C, OT], bf16, isOutput=False)
    wqL = nc.declare_dram_parameter("wqL", [NPAIR, P, NB_C * P], fp8, isOutput=False)
    wkL = nc.declare_dram_parameter("wkL", [NPAIR, P, NB_C * P], fp8, isOutput=False)
    wvL = nc.declare_dram_parameter("wvL", [N_HG, P, NB_C * HG * D], fp8, isOutput=False)
    w1 = nc.declare_dram_parameter("w1", [C, F], bf16, isOutput=False)
    w2 = nc.declare_dram_parameter("w2", [F, C], bf16, isOutput=False)
    b1r = nc.declare_dram_parameter("b1r", [P, NB_F], f32, isOutput=False)
    b2r = nc.declare_dram_parameter("b2r", [P, NB_C], f32, isOutput=False)
    g1r = nc.declare_dram_parameter("g1r", [P, NB_C], f32, isOutput=False)
    be1r = nc.declare_dram_parameter("be1r", [P, NB_C], f32, isOutput=False)
    g2r = nc.declare_dram_parameter("g2r", [P, NB_C], f32, isOutput=False)
    be2r = nc.declare_dram_parameter("be2r", [P, NB_C], f32, isOutput=False)
    cmask = nc.declare_dram_parameter("cmask", [P, 64], fp8, isOutput=False)
    outT = nc.declare_dram_parameter("outT", [C, OT], f32, isOutput=True)

    xP8_t = xP8[:].rearrange("j p (i t) -> j p i t", i=2)
    xoP8_t = xoP8[:].rearrange("j p (i t) -> j p i t", i=2)
    xTo16_t = xTo16[:].rearrange("(n p) t -> n p t", p=P)
    w1_t = w1[:].rearrange("(n p) f -> n p f", p=P)
    w2_t = w2[:].rearrange("(n p) c -> n p c", p=P)
    outT_t = outT[:].rearrange("(n p) t -> n p t", p=P)

    def r(ap):
        """bitcast fp32 matmul operands to fp32r (full-rate at N>=256)"""
        return ap.bitcast(f32r)

    with TileContext(nc) as tc, contextlib.ExitStack() as ctx:
        const = ctx.enter_context(tc.tile_pool(name="const", bufs=1))
        p_a = ctx.enter_context(tc.tile_pool(name="attn_a", bufs=1))
        p_mlpw = ctx.enter_context(tc.tile_pool(name="mlpw", bufs=1))
        import contextlib as _ctl
        att_stack = _ctl.ExitStack()
        p_xt = att_stack.enter_context(tc.tile_pool(name="xt", bufs=1))
        p_w = att_stack.enter_context(tc.tile_pool(name="wqkv", bufs=1))
        p_qk = att_stack.enter_context(tc.tile_pool(name="qk", bufs=1))
        p_v = att_stack.enter_context(tc.tile_pool(name="vv", bufs=1))
        p_ps = att_stack.enter_context(
            tc.tile_pool(name="ps", bufs=1, space="PSUM")
        )
        p_e = att_stack.enter_context(tc.tile_pool(name="et", bufs=1))
        p_dn = att_stack.enter_context(tc.tile_pool(name="dn", bufs=1))

        # ---- constants ----
        ones_k = const.tile([P, 1], bf16, tag="ones_k", name="ones_k")
        nc.vector.memset(ones_k, 1.0)
        stage_k = const.tile([P, 1], f32, tag="stage_k", name="stage_k")
        nc.vector.memset(stage_k, 1.0)
        stage_r = const.tile([1, P], f32, tag="stage_r", name="stage_r")
        nc.vector.memset(stage_r, 1.0)
        with nc.allow_low_precision(reason="f32r constants for bcast matmuls"):
            ones_k32 = const.tile([P, 1], f32r, tag="ones_k32", name="ones_k32")
            nc.vector.tensor_copy(ones_k32, stage_k)
            ones_row = const.tile([1, P], f32r, tag="ones_row", name="ones_row")
            nc.vector.tensor_copy(ones_row, stage_r)
            neg_row = const.tile([1, P], f32r, tag="neg_row", name="neg_row")
            nc.vector.tensor_scalar_mul(neg_row, stage_r, -1.0)
            negh_row = const.tile([1, P], f32r, tag="negh_row", name="negh_row")
            nc.vector.tensor_scalar_mul(negh_row, stage_r, -0.5)
        eps_t = const.tile([1, 1], f32, tag="eps", name="eps_t")
        nc.vector.memset(eps_t, EPS)
        msk = const.tile([P, 64], fp8, tag="msk", name="msk")
        nc.sync.dma_start(out=msk, in_=cmask[:])
        msk2 = bass.AP(
            tensor=msk.tensor, offset=msk.offset,
            ap=[list(msk.ap[0]), [0, 2], list(msk.ap[1])],
        )
        sb_b1 = const.tile([P, NB_F], f32, tag="b1", name="sb_b1")
        nc.sync.dma_start(out=sb_b1, in_=b1r[:])
        sb_b2 = const.tile([P, NB_C], f32, tag="b2", name="sb_b2")
        nc.sync.dma_start(out=sb_b2, in_=b2r[:])
        sb_g1 = const.tile([P, NB_C], f32, tag="g1", name="sb_g1")
        nc.sync.dma_start(out=sb_g1, in_=g1r[:])
        sb_be1 = const.tile([P, NB_C], f32, tag="be1", name="sb_be1")
        nc.sync.dma_start(out=sb_be1, in_=be1r[:])
        sb_g2 = const.tile([P, NB_C], f32, tag="g2", name="sb_g2")
        nc.sync.dma_start(out=sb_g2, in_=g2r[:])
        sb_be2 = const.tile([P, NB_C], f32, tag="be2", name="sb_be2")
        nc.sync.dma_start(out=sb_be2, in_=be2r[:])

        # ---- persistent activations (fp8 pair layout for DoubleRow) ----
        xp8 = [
            p_xt.tile([P, 2, T], fp8, tag=f"xp8_{j}", name=f"xp8_{j}")
            for j in range(N_HG)
        ]
        xo8 = [
            p_xt.tile([P, 2, OT], fp8, tag=f"xo8_{j}", name=f"xo8_{j}")
            for j in range(N_HG)
        ]
        sb_xto16 = []
        for k in range(NB_C):
            # xto16 lives in the persistent pool: the MLP-phase residual
            # adds still read it after the attention pools close.
            t3 = p_a.tile([P, OT], bf16, tag=f"xto16_{k}", name=f"xto16_{k}")
            nc.sync.dma_start(out=t3, in_=xTo16_t[k])
            sb_xto16.append(t3)
        # fine-grained loads: the first projection matmul only needs the
        # first 512-column slice of each pair tile.
        for t4 in range(T // 512):
            for j in range(N_HG):
                nc.sync.dma_start(
                    out=xp8[j][:, :, t4 * 512 : (t4 + 1) * 512],
                    in_=xP8_t[j][:, :, t4 * 512 : (t4 + 1) * 512],
                )
        for t2 in range(OT // 512):
            for j in range(N_HG):
                nc.sync.dma_start(
                    out=xo8[j][:, :, t2 * 512 : (t2 + 1) * 512],
                    in_=xoP8_t[j][:, :, t2 * 512 : (t2 + 1) * 512],
                )

        def xt_own(k, tb):
            """own-token columns of chunk k, token block tb."""
            return sb_xto16[k][:, tb * 512 : (tb + 1) * 512]

        # attention output a^T, bf16 [128, OT] per pair-chunk
        sb_a = [
            p_a.tile([P, OT], bf16, tag=f"a{pc}", name=f"a{pc}")
            for pc in range(NPAIR)
        ]

        # MLP weight tiles: allocate now (address space), DMA after the
        # first projection wave's weights are queued.
        sb_w1 = [
            p_mlpw.tile([P, F], bf16, tag=f"w1_{k}", name=f"w1_{k}")
            for k in range(NB_C)
        ]
        sb_w2 = [
            p_mlpw.tile([P, C], bf16, tag=f"w2_{m}", name=f"w2_{m}")
            for m in range(NB_F)
        ]

        # ============================================================
        # Phase A: attention, in head groups of HG. Pools are global so
        # group hg+1's projections overlap group hg's attention tail.
        # ============================================================
        q_t, k_t = {}, {}
        for hg in range(N_HG):
            pcs = [hg * (HG // 2) + i for i in range(HG // 2)]
            heads = [2 * pc + j for pc in pcs for j in range(2)]

            _sc_p = nc.enter_named_scope(f"proj{hg}", False)
            # ---- projections for this group ----
            for pc in pcs:
                q_t[pc] = p_qk.tile(
                    [P, T], bf16, tag=f"q{pc % 2}", bufs=2, name=f"q{pc}"
                )
                k_t[pc] = p_qk.tile(
                    [P, OT], bf16, tag=f"k{pc % 2}", bufs=2, name=f"k{pc}"
                )
                wqt = p_w.tile([P, NB_C * P], fp8, tag="wq", bufs=2, name="wqt")
                nc.sync.dma_start(out=wqt, in_=wqL[pc])
                for t4 in range(T // 512):
                    ps = p_ps.tile([P, 2, 512], f32, tag="ps", bufs=2, name="ps_prj")
                    for j in range(N_HG):
                        nc.tensor.matmul(
                            ps[:, 0, :],
                            wqt[:, 2 * j * P : 2 * (j + 1) * P].rearrange(
                                "p (i c) -> p i c", i=2
                            ),
                            xp8[j][:, :, t4 * 512 : (t4 + 1) * 512],
                            start=(j == 0),
                            stop=(j == N_HG - 1),
                            perf_mode=DR,
                        )
                    nc.vector.tensor_copy(
                        q_t[pc][:, t4 * 512 : (t4 + 1) * 512], ps[:, 0, :]
                    )
                wkt = p_w.tile([P, NB_C * P], fp8, tag="wk", bufs=2, name="wkt")
                nc.sync.dma_start(out=wkt, in_=wkL[pc])
                for t2 in range(OT // 512):
                    ps = p_ps.tile([P, 2, 512], f32, tag="ps", bufs=2, name="ps_prk")
                    for j in range(N_HG):
                        nc.tensor.matmul(
                            ps[:, 0, :],
                            wkt[:, 2 * j * P : 2 * (j + 1) * P].rearrange(
                                "p (i c) -> p i c", i=2
                            ),
                            xo8[j][:, :, t2 * 512 : (t2 + 1) * 512],
                            start=(j == 0),
                            stop=(j == N_HG - 1),
                            perf_mode=DR,
                        )
                    nc.vector.tensor_copy(
                        k_t[pc][:, t2 * 512 : (t2 + 1) * 512], ps[:, 0, :]
                    )

            # v projection: full T, DoubleRow, into fp8 [128, 2, HG, 65]
            # s-chunk-pair tiles (the AV DoubleRow stationary layout).
            wvt = p_w.tile(
                [P, NB_C * HG * D], fp8, tag="wv", bufs=2, name="wvt"
            )
            nc.sync.dma_start(out=wvt, in_=wvL[hg])
            v8 = []
            for jp in range(T // P // 2):
                # head slot padded 65 -> 68 so the DoubleRow pair stride
                # (HG*68 = 272 B) meets the 16 B LDWEIGHTS alignment rule.
                vt = p_v.tile(
                    [P, 2, HG, 68], fp8, tag=f"v8_{jp}", bufs=2, name=f"v8_{jp}"
                )
                nc.vector.memset(vt[:, :, :, 64:65], 1.0)
                v8.append(vt)
            for sc in range(T // P):
                ps = p_ps.tile([P, 2, 512], f32, tag="ps", bufs=2, name="ps_v")
                for j in range(N_HG):
                    nc.tensor.matmul(
                        ps[:, 0, 0 : HG * D],
                        xp8[j][:, :, sc * P : (sc + 1) * P],
                        wvt[:, 2 * j * HG * D : 2 * (j + 1) * HG * D].rearrange(
                            "p (i c) -> p i c", i=2
                        ),
                        start=(j == 0),
                        stop=(j == N_HG - 1),
                        perf_mode=DR,
                    )
                nc.scalar.activation(
                    out=v8[sc // 2][:, sc % 2, :, 0:64],
                    in_=ps[:, 0, 0 : HG * D].rearrange("p (h d) -> p h d", h=HG),
                    func=AF.Identity,
                    scale=0.125,
                )

            nc.leave_named_scope(f"proj{hg}", _sc_p[0], False)
            if hg == 0:
                for k in range(NB_C):
                    nc.sync.dma_start(out=sb_w1[k], in_=w1_t[k])
                for m in range(NB_F):
                    nc.sync.dma_start(out=sb_w2[m], in_=w2_t[m])
            _sc_a = nc.enter_named_scope(f"attn{hg}", False)
            # ---- attention ----
            for tb in range(2):
                nsc = 8 * tb + 8          # s-chunks for this own-block
                av = {}
                for h in heads:
                    av[h] = p_ps.tile(
                        [P, 512], f32, tag=f"av{h % HG}", bufs=1, name=f"av{h}"
                    )
                npp = nsc // 2
                for jp in range(npp):
                    sc0, sc1 = 2 * jp, 2 * jp + 1
                    c0e = max(0, 64 * sc0 - 512 * tb)
                    c0o = max(0, 64 * sc1 - 512 * tb)
                    for pc in pcs:
                        et8 = p_e.tile(
                            [P, 2, 2, 512], fp8, tag="exp", bufs=4, name="et8"
                        )
                        for i, (sc, c0) in enumerate(
                            ((sc0, c0e), (sc1, c0o))
                        ):
                            ps = p_ps.tile(
                                [P, 2, 512], f32, tag="ps", bufs=2, name="ps_sc"
                            )
                            for par in range(2):
                                nc.tensor.matmul(
                                    ps[:, par, c0:512],
                                    q_t[pc][par * 64 : par * 64 + 64,
                                            sc * P : (sc + 1) * P],
                                    k_t[pc][par * 64 : par * 64 + 64,
                                            tb * 512 + c0 : (tb + 1) * 512],
                                    start=True,
                                    stop=True,
                                )
                            nc.scalar.activation(
                                out=et8[:, :, i, c0:512],
                                in_=ps[:, :, c0:512],
                                func=AF.Exp,
                                scale=SCALE / 64.0,
                            )
                        if sc0 >= 8 * tb:   # causal boundary stripes
                            nc.vector.tensor_tensor(
                                et8[:, :, 0, c0e : c0e + 64],
                                et8[:, :, 0, c0e : c0e + 64],
                                msk2[:, :, 0:64],
                                ALU.mult,
                            )
                            nc.vector.memset(
                                et8[:, :, 1, c0e : c0e + 64], 0.0
                            )
                            nc.vector.tensor_tensor(
                                et8[:, :, 1, c0o : c0o + 64],
                                et8[:, :, 1, c0o : c0o + 64],
                                msk2[:, :, 0:64],
                                ALU.mult,
                            )
                        for par in range(2):
                            h = 2 * pc + par
                            jj = heads.index(h)
                            nc.tensor.matmul(
                                av[h][0:65, c0e:512],
                                v8[jp][:, :, jj, 0:65],
                                et8[:, par, :, c0e:512],
                                start=(jp == 0),
                                stop=(jp == npp - 1),
                                perf_mode=DR,
                            )
                # normalize: 1/den = exp(-ln den), negation folded into
                # the broadcast matmul; single multiply reads AV psum.
                for h in heads:
                    pc, par = h // 2, h % 2
                    nl = p_dn.tile([1, 512], f32r, tag="nl", bufs=4, name="nl")
                    nc.scalar.activation(
                        out=nl, in_=av[h][64:65, 0:512], func=AF.Ln
                    )
                    rb = p_ps.tile(
                        [P, 2, 512], f32, tag="ps", bufs=2, name="rb"
                    )
                    nc.tensor.matmul(
                        rb[0:64, 0, :], neg_row[:, 0:64], nl,
                        start=True, stop=True,
                    )
                    rec = p_dn.tile(
                        [64, 512], f32, tag="rec", bufs=4, name="rec"
                    )
                    nc.scalar.activation(
                        out=rec, in_=rb[0:64, 0, :], func=AF.Exp
                    )
                    nc.vector.tensor_tensor(
                        sb_a[pc][par * 64 : par * 64 + 64,
                                 tb * 512 : (tb + 1) * 512],
                        av[h][0:64, 0:512],
                        rec,
                        ALU.mult,
                    )

            nc.leave_named_scope(f"attn{hg}", _sc_a[0], False)

        att_stack.close()   # free attention pools before the MLP pools open

        # ============================================================
        # Phase B: residual + LN1 + MLP + residual + LN2
        # ============================================================
        with contextlib.ExitStack() as mctx:
            mctx.enter_context(nc.named_scope("mlp"))
            p_r1 = mctx.enter_context(tc.tile_pool(name="r1", bufs=1))
            p_ln = mctx.enter_context(tc.tile_pool(name="ln", bufs=1))
            p_tmp = mctx.enter_context(tc.tile_pool(name="tmp", bufs=1))
            p_st = mctx.enter_context(tc.tile_pool(name="st", bufs=1))
            p_psm = mctx.enter_context(
                tc.tile_pool(name="psm", bufs=1, space="PSUM")
            )
            p_h = mctx.enter_context(tc.tile_pool(name="hsb", bufs=1))
            p_out = mctx.enter_context(tc.tile_pool(name="outp", bufs=1))

            r1 = [
                p_r1.tile([P, OT], f32r, tag=f"r1_{c}", name=f"r1_{c}")
                for c in range(NB_C)
            ]
            ln1 = [
                p_ln.tile([P, OT], bf16, tag=f"ln1_{c}", name=f"ln1_{c}")
                for c in range(NB_C)
            ]

            def layer_norm_T(src_tiles, out_cb):
                """transposed LN over the partition (c) dim. Stats via
                f32r/bf16 ones-matmuls; rsqrt as exp(-0.5 ln(var+eps));
                broadcasts via rank-1 matmuls. src_tiles: 6 x [128,512]
                f32 SBUF views."""
                mu_ps = p_psm.tile([1, 512], f32, tag="lnst", bufs=2, name="mu_ps")
                sq_ps = p_psm.tile([1, 512], f32, tag="lnst", bufs=2, name="sq_ps")
                for c in range(NB_C):
                    s = p_tmp.tile([P, 512], bf16, tag="sqt", bufs=1, name="sqt")
                    nc.vector.tensor_tensor(s, src_tiles[c], src_tiles[c], ALU.mult)
                    nc.tensor.matmul(
                        mu_ps, ones_k32, src_tiles[c],
                        start=(c == 0), stop=(c == NB_C - 1),
                    )
                    nc.tensor.matmul(
                        sq_ps, ones_k, s,
                        start=(c == 0), stop=(c == NB_C - 1),
                    )
                mun = p_st.tile([1, 512], f32r, tag="mun", bufs=1, name="mun")
                nc.scalar.activation(
                    out=mun, in_=mu_ps, func=AF.Identity, scale=1.0 / C
                )
                m2 = p_st.tile([1, 512], f32, tag="m2", bufs=1, name="m2")
                nc.scalar.activation(
                    out=m2, in_=mu_ps, func=AF.Square, scale=1.0 / C
                )
                var = p_st.tile([1, 512], f32, tag="var", bufs=1, name="var")
                nc.vector.tensor_scalar_mul(var, sq_ps, 1.0 / C)
                nc.vector.tensor_tensor(var, var, m2, ALU.subtract)
                lv = p_st.tile([1, 512], f32r, tag="lv", bufs=1, name="lv")
                nc.scalar.activation(
                    out=lv, in_=var, func=AF.Ln, bias=eps_t, scale=1.0
                )
                mu_b = p_psm.tile([P, 512], f32, tag="lnbc", bufs=2, name="mu_b")
                nc.tensor.matmul(mu_b, ones_row, mun, start=True, stop=True)
                rs_ps = p_psm.tile([P, 512], f32, tag="lnbc", bufs=2, name="rs_ps")
                nc.tensor.matmul(rs_ps, negh_row, lv, start=True, stop=True)
                rs_b = p_tmp.tile([P, 512], f32, tag="rs_b", bufs=2, name="rs_b")
                nc.scalar.activation(out=rs_b, in_=rs_ps, func=AF.Exp)
                for c in range(NB_C):
                    d1 = p_tmp.tile([P, 512], f32, tag="d1", bufs=2, name="d1")
                    nc.vector.tensor_tensor(d1, src_tiles[c], mu_b, ALU.subtract)
                    nc.vector.tensor_tensor(d1, d1, rs_b, ALU.mult)
                    out_cb(c, d1)

            # residual + LN1 for BOTH halves first: LN1(tb=1)'s DVE work
            # then overlaps MLP(tb=0)'s matmuls.
            for tb in range(2):
                sl = slice(tb * 512, (tb + 1) * 512)
                r1v = []
                for c in range(NB_C):
                    with nc.allow_low_precision(reason="f32r residual store"):
                        nc.vector.tensor_tensor(
                            r1[c][:, sl], xt_own(c, tb), sb_a[c][:, sl], ALU.add
                        )
                    r1v.append(r1[c][:, sl])

                def ln1_out(c, d2, _sl=sl):
                    nc.vector.tensor_scalar(
                        out=ln1[c][:, _sl], in0=d2,
                        scalar1=sb_g1[:, c : c + 1], scalar2=sb_be1[:, c : c + 1],
                        op0=ALU.mult, op1=ALU.add,
                    )
                    # f32 copy for the post-LN residual (r1 is dead: reuse)
                    with nc.allow_low_precision(reason="f32r residual store"):
                        nc.vector.tensor_scalar(
                            out=r1[c][:, _sl], in0=d2,
                            scalar1=sb_g1[:, c : c + 1], scalar2=sb_be1[:, c : c + 1],
                            op0=ALU.mult, op1=ALU.add,
                        )

                layer_norm_T(r1v, ln1_out)

            # MLP pass 1, both halves per m-tile: the stationary W1 slice
            # is shared by consecutive matmuls (halves the LDWEIGHTS
            # serialization) and gelu covers [P, OT] per tile.
            h_sb = []
            for m in range(NB_F):
                hp = {}
                for tb in range(2):
                    hp[tb] = p_psm.tile(
                        [P, 512], f32, tag="h_ps", bufs=2, name="h_ps"
                    )
                for k in range(NB_C):
                    for tb in range(2):
                        nc.tensor.matmul(
                            hp[tb],
                            sb_w1[k][:, m * P : (m + 1) * P],
                            ln1[k][:, tb * 512 : (tb + 1) * 512],
                            start=(k == 0),
                            stop=(k == NB_C - 1),
                        )
                hs = p_h.tile([P, OT], bf16, tag=f"h{m}", name=f"h{m}")
                for tb in range(2):
                    nc.scalar.activation(
                        out=hs[:, tb * 512 : (tb + 1) * 512], in_=hp[tb],
                        func=AF.Gelu, bias=sb_b1[:, m : m + 1], scale=1.0,
                    )
                h_sb.append(hs)

            for tb in range(2):
                sl = slice(tb * 512, (tb + 1) * 512)

                # MLP pass 2: y[c] = sum_m W2[m,c].T h[m];  r2 = y + b2 + r1
                r2v = []
                for c in range(NB_C):
                    y_ps = p_psm.tile([P, 512], f32, tag="y_ps", bufs=2, name="y_ps")
                    for m in range(NB_F):
                        nc.tensor.matmul(
                            y_ps,
                            sb_w2[m][:, c * P : (c + 1) * P],
                            h_sb[m][:, sl],
                            start=(m == 0),
                            stop=(m == NB_F - 1),
                        )
                    y_sb = p_tmp.tile([P, 512], f32, tag="y_sb", bufs=2, name="y_sb")
                    nc.vector.tensor_scalar(
                        out=y_sb, in0=y_ps,
                        scalar1=sb_b2[:, c : c + 1], scalar2=None,
                        op0=ALU.add,
                    )
                    with nc.allow_low_precision(reason="f32r residual store"):
                        nc.vector.tensor_tensor(
                            r1[c][:, sl], y_sb, r1[c][:, sl], ALU.add
                        )
                    r2v.append(r1[c][:, sl])

                def ln2_out(c, d2, _sl=sl):
                    o = p_out.tile([P, 512], f32, tag="o", bufs=2, name="o")
                    nc.vector.tensor_scalar(
                        out=o, in0=d2,
                        scalar1=sb_g2[:, c : c + 1], scalar2=sb_be2[:, c : c + 1],
                        op0=ALU.mult, op1=ALU.add,
                    )
                    nc.sync.dma_start(out=outT_t[c][:, _sl], in_=o)

                layer_norm_T(r2v, ln2_out)

    return nc


def _spill_excess_waits(nc, maxw=2):
    """walrus (this build) caps sync-wait commands per instruction. Move
    excess waits onto freshly inserted same-engine nops placed immediately
    before the over-limit instruction (same engine stream => the waits
    still complete before it executes)."""
    import copy

    import concourse.bass as bass
    import concourse.mybir as mybir

    scratch = bass.Bass()
    tpl = scratch.sync.nop(nofuse=True).ins
    ctr = [0]

    def mknop(engine, waits):
        n = copy.deepcopy(tpl)
        ctr[0] += 1
        n.name = f"I-spill{ctr[0]}"
        n.engine = engine
        n.sync_info = mybir.SyncInfo(on_wait=list(waits), on_update=[])
        return n

    fn = nc.m.functions[0]
    for bb in fn.blocks:
        changed = False
        out = []
        for inst in bb.instructions:
            si = inst.sync_info
            waits = list(si.on_wait) if si and si.on_wait else []
            nupd = len(si.on_update) if si and si.on_update else 0
            lim = max(0, maxw - nupd)   # waits + updates <= maxw total
            if len(waits) > lim:
                keep = waits[-lim:] if lim else []
                rest = waits[: len(waits) - lim]
                while rest:
                    chunk, rest = rest[:1], rest[1:]
                    out.append(mknop(inst.engine, chunk))
                si.on_wait = keep
                changed = True
            out.append(inst)
        if changed:
            bb.instructions = out


def _get_nc():
    if "nc" not in _compiled:
        _patch_tile_drain()
        _patch_profile_hook()
        nc = _build_nc()
        _spill_excess_waits(nc, maxw=2)
        _compiled["nc"] = nc
    return _compiled["nc"]


# --------------------------------------------------------------------------
# host-side sharding
# --------------------------------------------------------------------------

E4 = ml_dtypes.float8_e4m3


def _q8(a):
    return np.clip(a, -240.0, 240.0).astype(E4)


def _make_in_maps(x, Wq, Wk, Wv, ln1_g, ln1_b, W1, b1, W2, b2, ln2_g, ln2_b):
    x = np.asarray(x, np.float32)
    wq_s = np.ascontiguousarray(
        np.asarray(Wq, np.float32).transpose(1, 0, 2).reshape(C, C)
    )
    wk_s = np.ascontiguousarray(
        np.asarray(Wk, np.float32).transpose(1, 0, 2).reshape(C, C)
    )
    wv_s = np.ascontiguousarray(
        np.asarray(Wv, np.float32).transpose(1, 0, 2).reshape(C, C)
    )
    # fp8 DoubleRow pair layouts (weights pre-scaled by 16, x by 1/2;
    # the 64x score factor is folded into the exp scale, the 8x v factor
    # into the v-copy scale).
    # wqL[pc, p, j*256 + i*128 + c2] = 16*wq[(2j+i)*128+p, pc*128+c2]
    wqL = np.ascontiguousarray(
        _q8(wq_s.reshape(N_HG, 2, P, NPAIR, P).transpose(3, 2, 0, 1, 4) * 16.0)
        .reshape(NPAIR, P, C)
    )
    wkL = np.ascontiguousarray(
        _q8(wk_s.reshape(N_HG, 2, P, NPAIR, P).transpose(3, 2, 0, 1, 4) * 16.0)
        .reshape(NPAIR, P, C)
    )
    # wvL[hg, p, j*512 + i*256 + c2] = 16*wv[(2j+i)*128+p, hg*256+c2]
    wvL = np.ascontiguousarray(
        _q8(wv_s.reshape(N_HG, 2, P, N_HG, HG * D).transpose(3, 2, 0, 1, 4) * 16.0)
        .reshape(N_HG, P, NB_C * HG * D)
    )
    w1b = np.asarray(W1, np.float32).astype(BF16)
    w2b = np.asarray(W2, np.float32).astype(BF16)
    b1r = np.ascontiguousarray(np.asarray(b1, np.float32).reshape(NB_F, P).T)
    b2r = np.ascontiguousarray(np.asarray(b2, np.float32).reshape(NB_C, P).T)
    g1r = np.ascontiguousarray(np.asarray(ln1_g, np.float32).reshape(NB_C, P).T)
    be1r = np.ascontiguousarray(np.asarray(ln1_b, np.float32).reshape(NB_C, P).T)
    g2r = np.ascontiguousarray(np.asarray(ln2_g, np.float32).reshape(NB_C, P).T)
    be2r = np.ascontiguousarray(np.asarray(ln2_b, np.float32).reshape(NB_C, P).T)

    in_maps = []
    for core in range(N_CORES):
        b, g = core // 2, core % 2
        xb = x[b]                                # [T, C]
        xTa = np.ascontiguousarray(xb.T)         # [C, T]
        own = np.arange(g, T, 2)
        xo = np.ascontiguousarray(xb[own].T)     # [C, OT]
        # fp8 pair layouts: xP8[j, p, i*T + t] = fp8(0.5*x[t, (2j+i)*128+p])
        xP8 = np.ascontiguousarray(
            _q8(xTa.reshape(N_HG, 2, P, T).transpose(0, 2, 1, 3) * 0.5)
            .reshape(N_HG, P, 2 * T)
        )
        xoP8 = np.ascontiguousarray(
            _q8(xo.reshape(N_HG, 2, P, OT).transpose(0, 2, 1, 3) * 0.5)
            .reshape(N_HG, P, 2 * OT)
        )
        ii = np.arange(P)[:, None]
        mm = np.arange(64)[None, :]
        cm = np.where(ii <= 2 * mm + g, 1.0, 0.0).astype(E4)
        in_maps.append(
            {
                "xP8": xP8,
                "xoP8": xoP8,
                "xTo16": xo.astype(BF16),
                "wqL": wqL,
                "wkL": wkL,
                "wvL": wvL,
                "w1": w1b,
                "w2": w2b,
                "b1r": b1r,
                "b2r": b2r,
                "g1r": g1r,
                "be1r": be1r,
                "g2r": g2r,
                "be2r": be2r,
                "cmask": cm,
            }
        )
    return in_maps


def _assemble(results):
    out = np.empty((B, T, C), np.float32)
    for core in range(N_CORES):
        b, g = core // 2, core % 2
        own = np.arange(g, T, 2)
        out[b, own, :] = results[core]["outT"].T
    return out


def kernel(_trace=False, **inputs):
    from concourse.bass_utils import run_bass_kernel_spmd

    nc = _get_nc()
    in_maps = _make_in_maps(**inputs)
    res = run_bass_kernel_spmd(nc, in_maps, list(range(N_CORES)), trace=_trace)
    out = _assemble(res.results)
    if _trace:
        return out, res
    return out
